# revision 5
# baseline (speedup 1.0000x reference)
import sys
sys.path.insert(0, "/opt/trn_rl_repo")
import numpy as np
import ml_dtypes
import concourse.bacc as bacc
import concourse.bass as bass
import concourse.mybir as mybir
import concourse.tile as tile
from concourse.bass import ds, ts
from concourse.bass_utils import run_bass_kernel_spmd

BF = ml_dtypes.bfloat16
P = 128
NT = 577
D = 768
H = 16
KO = 7            # 896 = 7*128 contraction tiles (768 x-dims + bias row + pad)
NBLK = [(0, 128), (128, 128), (256, 128), (384, 128), (512, 65)]
MPAD = 580        # m padded to 4*145
NG = 145          # m4 groups
SCALE = 48 ** -0.5

_cache = {}


def _build(nc):
    bf = mybir.dt.bfloat16
    f32 = mybir.dt.float32
    EXPF = mybir.ActivationFunctionType.Exp
    x_d = nc.dram_tensor("xT", [896, NT], bf, kind="ExternalInput")
    xf_d = nc.dram_tensor("xfT", [896, NT], bf, kind="ExternalInput")
    wq_d = nc.dram_tensor("wqT", [768, 1024], bf, kind="ExternalInput")
    wk_d = nc.dram_tensor("wkT", [768, 1024], bf, kind="ExternalInput")
    wv_d = nc.dram_tensor("wvT", [896, 784], bf, kind="ExternalInput")
    bq_d = nc.dram_tensor("bqT", [P, 8], f32, kind="ExternalInput")
    pw_d = nc.dram_tensor("pwT", [896, D], bf, kind="ExternalInput")
    wb_d = nc.dram_tensor("wblk", [P, 64], bf, kind="ExternalInput")
    id_d = nc.dram_tensor("idn", [P, P], bf, kind="ExternalInput")
    out_d = nc.dram_tensor("outT", [D, NT], f32, kind="ExternalOutput")

    with tile.TileContext(nc) as tc:
        wp = tc.alloc_tile_pool(name="wp", bufs=1)
        bk = tc.alloc_tile_pool(name="bk", bufs=2)
        ps = tc.alloc_tile_pool(name="ps", bufs=2, space="PSUM")
        ld2 = tc.alloc_tile_pool(name="ld2", bufs=1)   # lives until cycle 1 done
        ld = tc.alloc_tile_pool(name="ld", bufs=1)     # released after phase 1

        # persistent tiles
        wblk = wp.tile([P, 64], bf)
        idn = wp.tile([P, P], bf)
        bq = wp.tile([P, 8], f32)
        qt = wp.tile([P, 8, 640], bf)
        qft = wp.tile([P, 8, 640], bf)
        kt = wp.tile([P, 8, 584], bf)
        kft = wp.tile([P, 8, 584], bf)
        v_sb = wp.tile([P, 5, 784], bf)
        pw = wp.tile([P, KO, D], bf)
        wt = wp.tile([P, KO, 640], bf)
        A1 = wp.tile([P, MPAD, 32], bf)
        E2 = wp.tile([P, 16, 640], bf)

        # load-phase tiles; DMA order chosen so compute can start early
        wq = ld.tile([P, 6, 1024], bf, bufs=1)
        wk = ld.tile([P, 6, 1024], bf, bufs=1)
        xf_sb = ld.tile([P, KO, 584], bf, bufs=1)
        wv = ld2.tile([P, KO, 784], bf, bufs=1)
        x_sb = ld2.tile([P, KO, 584], bf, bufs=1)
        nc.sync.dma_start(x_sb[:, :, :NT], x_d.rearrange("(ko p) m -> p ko m", p=P))
        nc.sync.dma_start(wk[:], wk_d.rearrange("(ko p) m -> p ko m", p=P))
        nc.sync.dma_start(wq[:], wq_d.rearrange("(ko p) m -> p ko m", p=P))
        nc.sync.dma_start(bq[:], bq_d[:])
        nc.sync.dma_start(xf_sb[:, :, :NT], xf_d.rearrange("(ko p) m -> p ko m", p=P))
        nc.sync.dma_start(wv[:], wv_d.rearrange("(ko p) m -> p ko m", p=P))
        nc.sync.dma_start(idn[:], id_d[:])
        nc.sync.dma_start(wblk[:], wb_d[:])
        nc.sync.dma_start(pw[:], pw_d.rearrange("(ko p) m -> p ko m", p=P))

        nc.vector.memset(qt[:, :, NT:640], 0.0)
        nc.vector.memset(qft[:, :, NT:640], 0.0)
        nc.vector.memset(A1[:, NT:MPAD, :], 0.0)
        nc.vector.memset(E2[:, :, MPAD:640], 0.0)
        nc.vector.memset(wt[:, KO - 1, :], 0.0)
        nc.vector.memset(wt[0:1, KO - 1, :], 1.0)

        As = [A1, None]      # second A allocated after ld release

        # ---- qkv projection helpers ----
        def kq_proj(w_sb, dst, src, t, bias):
            ppA = ps.tile([P, 512], f32, tag="rp", name="ppA")
            ppB = ps.tile([P, 65], f32, tag="mp", name="ppB")
            for ko in range(6):
                nc.tensor.matmul(ppA[:, :], w_sb[:, ko, ts(t, P)],
                                 src[:, ko, ds(0, 512)],
                                 start=(ko == 0), stop=(ko == 5))
            for ko in range(6):
                nc.tensor.matmul(ppB[:, :], w_sb[:, ko, ts(t, P)],
                                 src[:, ko, ds(512, 65)],
                                 start=(ko == 0), stop=(ko == 5))
            if bias is None:
                nc.vector.tensor_copy(dst[:, t, 0:512], ppA[:, :])
                nc.vector.tensor_copy(dst[:, t, 512:NT], ppB[:, :])
            else:
                nc.vector.tensor_scalar_add(dst[:, t, 0:512], ppA[:, :], bias)
                nc.vector.tensor_scalar_add(dst[:, t, 512:NT], ppB[:, :], bias)

        def v_chunk(mt):
            m0, mlen = NBLK[mt]
            ppA = ps.tile([P, 512], f32, tag="rp", name="vpA")
            ppB = ps.tile([P, 272], f32, tag="mp", name="vpB")
            for ko in range(KO):
                nc.tensor.matmul(ppA[:mlen, :], x_sb[:, ko, ds(m0, mlen)],
                                 wv[:, ko, ds(0, 512)],
                                 start=(ko == 0), stop=(ko == KO - 1))
            for ko in range(KO):
                nc.tensor.matmul(ppB[:mlen, :], x_sb[:, ko, ds(m0, mlen)],
                                 wv[:, ko, ds(512, 272)],
                                 start=(ko == 0), stop=(ko == KO - 1))
            nc.vector.tensor_copy(v_sb[:mlen, mt, 0:512], ppA[:mlen, :])
            nc.vector.tensor_copy(v_sb[:mlen, mt, 512:784], ppB[:mlen, :])

        # ---- per-block score channel ----
        def score_c(bi, c, zt_k):
            n0, nlen = NBLK[bi]
            A = As[bi % 2]
            qs, ks_ = (qt, kt) if c < 16 else (qft, kft)
            h = c % 16
            t, off = h // 2, 64 * (h % 2)
            sp = ps.tile([P, 784], f32, tag="sp", name="sp")
            for o0, w in ((0, 512), (512, 65)):
                nc.tensor.matmul(sp[:, o0:o0 + w], qs[off:off + 64, t, ds(n0, P)],
                                 ks_[off:off + 64, t, ds(o0, w)],
                                 start=True, stop=True)
            nc.scalar.activation(A[:, :NT, c], sp[:, :NT], EXPF, scale=SCALE,
                                 accum_out=zt_k[:, c:c + 1])

        # ---- pv stage: (og, mt)-granular transposes into grouped e2t tiles ----
        def make_pv(w_acc):
            """Returns (tpiece, mpiece, norm) closures for one block's pv."""
            e2tgs = {}
            boxes = {}

            def tpiece(og, mt):
                tp = ps.tile([P, 4, P], bf, tag="rp", name="tp")
                for oj in range(4):
                    o = 4 * og + oj
                    nc.tensor.transpose(tp[:, oj, :], E2[:, o, ds(P * mt, P)], idn[:])
                if mt == 0:
                    e2tgs[og] = bk.tile([P, 4, 5, P], bf, tag="e2tg", bufs=3,
                                        name="e2tg")
                nc.vector.tensor_copy(e2tgs[og][:, :, mt, :], tp[:, :, :])

            def mpiece(og, oj):
                if og not in boxes:
                    boxes[og] = ps.tile([P, 4, 49], f32, tag="mp", name="pv4")
                pv4 = boxes[og]
                o = 4 * og + oj
                for mt, (m0, mlen) in enumerate(NBLK):
                    nc.tensor.matmul(pv4[:, oj, :], e2tgs[og][:mlen, oj, mt, :],
                                     v_sb[:mlen, mt, ds(49 * o, 49)],
                                     start=(mt == 0), stop=(mt == 4))

            def norm(og):
                pv4 = boxes[og]
                zr4 = bk.tile([P, 4], f32, tag="zr4", bufs=2, name="zr4")
                nc.vector.reciprocal(zr4[:], pv4[:, :, 48])
                nc.vector.tensor_mul(w_acc[:, ds(4 * og, 4), :], pv4[:, :, :48],
                                     zr4.unsqueeze(2).broadcast_to([P, 4, 48]))

            return tpiece, mpiece, norm

        def pv_pieces(bi, w_acc):
            tpiece, mpiece, norm = make_pv(w_acc)
            for og in range(4):
                for mt in range(5):
                    yield lambda og=og, mt=mt: tpiece(og, mt)
                yield lambda og=og: mpiece(og, 0)
                yield lambda og=og: mpiece(og, 1)
                yield lambda og=og: mpiece(og, 2)
                yield lambda og=og: (mpiece(og, 3), norm(og))

        def wt_pieces(bi, w_acc):
            n0, nlen = NBLK[bi]
            wa = w_acc.rearrange("p o d -> p (o d)")

            def one(j):
                wpp = ps.tile([P, P], bf, tag="rp", name="wpp")
                nc.tensor.transpose(wpp[:, :nlen], wa[:nlen, ds(P * j, P)],
                                    idn[:nlen, :nlen])
                nc.vector.tensor_copy(wt[:, j, n0:n0 + nlen], wpp[:, :nlen])
            for j in range(0, 6, 2):
                yield lambda j=j: (one(j), one(j + 1))

        def proj_pieces(bi):
            n0, nlen = NBLK[bi]

            def one(dt):
                fp = ps.tile([P, P], f32, tag="mp", name="fpj")
                for ko in range(KO):
                    nc.tensor.matmul(fp[:, :nlen], pw[:, ko, ts(dt, P)],
                                     wt[:, ko, ds(n0, nlen)],
                                     start=(ko == 0), stop=(ko == KO - 1))
                ob = bk.tile([P, P], f32, tag="ob", bufs=2, name="ob")
                nc.vector.tensor_copy(ob[:, :nlen], fp[:, :nlen])
                nc.sync.dma_start(out_d[ts(dt, P), ds(n0, nlen)], ob[:, :nlen])
            for dt in range(6):
                yield lambda dt=dt: one(dt)

        # ---- mix pieces (one per 8-group chunk, mm lagged one chunk) ----
        def mix_pieces(bi, zi_box):
            A = As[bi % 2]
            E2v = E2[:, :, :MPAD].rearrange("p o (g mj) -> p g mj o", mj=4)
            st = {"prev": None}

            def emit_mm(rs, gq, ng):
                mp = ps.tile([P, 8, 64], f32, tag="mp", name="mp")
                for gi in range(ng):
                    nc.tensor.matmul(mp[:, gi, :], rs[:, gi, :], wblk[:],
                                     start=True, stop=True)
                mpv = mp.rearrange("p g (mj o) -> p g mj o", o=16)
                nc.scalar.activation(E2v[:, ds(gq, ng), :, :], mpv[:, :ng, :, :], EXPF)

            def chunk(gq, ng):
                rp = ps.tile([P, 8, P], bf, tag="rp", name="rp")
                for gi in range(ng):
                    slab = A[:, ds(4 * (gq + gi), 4), :].rearrange("p m c -> p (m c)")
                    nc.tensor.transpose(rp[:, gi, :], slab, idn[:])
                rs = bk.tile([P, 8, P], bf, tag="rs", bufs=3, name="rs")
                nc.vector.tensor_mul(rs[:, :ng, :], rp[:, :ng, :],
                                     zi_box[0].unsqueeze(1).broadcast_to([P, ng, P]))
                if st["prev"] is not None:
                    emit_mm(*st["prev"])
                st["prev"] = (rs, gq, ng)

            for gq in range(0, NG, 8):
                yield lambda gq=gq: chunk(gq, min(8, NG - gq))
            yield lambda: emit_mm(*st["prev"])

        def zi_chain(zt_k, zi_box):
            zr_k = wp.tile([P, 32], bf, tag="zr", bufs=2, name="zr")
            with nc.allow_low_precision(reason="bf16 recip of softmax denom"):
                nc.vector.reciprocal(zr_k[:], zt_k[:])
            zpT = ps.tile([P, P], bf, tag="rp", name="zpT")
            for mj in range(4):
                nc.tensor.matmul(zpT[32 * mj:32 * mj + 32, :], zr_k[:, :],
                                 idn[:], is_transpose=True,
                                 tile_position=(0, 32 * mj),
                                 skip_group_check=True)
            zi_rep = bk.tile([P, P], bf, tag="zi", bufs=2, name="zi")
            nc.vector.tensor_copy(zi_rep[:], zpT[:])
            zi_box[0] = zi_rep

        # ================= emission =================
        # phase 1: k/q projections with block-0 scores interleaved.
        zt_ks = [wp.tile([P, 32], f32, tag="zt", bufs=2, name="zt0")]
        for t in range(8):
            kq_proj(wk, kt, x_sb, t, None)
            kq_proj(wq, qt, x_sb, t, bq[:, t:t + 1])
            score_c(0, 2 * t, zt_ks[0])
            score_c(0, 2 * t + 1, zt_ks[0])
        for t in range(8):
            kq_proj(wk, kft, xf_sb, t, None)
            kq_proj(wq, qft, xf_sb, t, bq[:, t:t + 1])
            score_c(0, 16 + 2 * t, zt_ks[0])
            score_c(0, 16 + 2 * t + 1, zt_ks[0])

        # wq/wk/xf dead from here; reuse their space for the second A
        ld.release()
        db = tc.alloc_tile_pool(name="db", bufs=1)
        A2 = db.tile([P, MPAD, 32], bf)
        nc.vector.memset(A2[:, NT:MPAD, :], 0.0)
        As[1] = A2

        # cycles: block bi scores/exp1 overlap mix(bi-1) then pv/wt/proj(bi-1).
        # v projections ride in cycle 1's early window (PE slack there).
        for bi in range(1, 5):
            zt_k = wp.tile([P, 32], f32, tag="zt", bufs=2, name="zt")
            zt_ks.append(zt_k)
            w_acc = bk.tile([P, 16, 48], bf, tag="wacc", bufs=2, name="wacc")
            zi_box = [None]
            zi_chain(zt_ks[bi - 1], zi_box)
            early = list(mix_pieces(bi - 1, zi_box))           # 20 pieces
            if bi == 1:
                vs = [lambda mt=mt: v_chunk(mt) for mt in range(5)]
                early = [p for pair in zip(early[:5], vs) for p in pair] + early[5:]
            late = list(pv_pieces(bi - 1, w_acc))              # 36 pieces
            late += list(wt_pieces(bi - 1, w_acc))             # 3
            late += list(proj_pieces(bi - 1))                  # 6
            NE = len(early)
            NL = len(late)
            edone = ldone = 0
            for c in range(32):
                score_c(bi, c, zt_k)
                if c < 20:
                    want = (c + 1) * NE // 20
                    while edone < want:
                        early[edone]()
                        edone += 1
                else:
                    want = (c - 19) * NL // 12
                    while ldone < want:
                        late[ldone]()
                        ldone += 1
            while edone < NE:
                early[edone]()
                edone += 1
            while ldone < NL:
                late[ldone]()
                ldone += 1

        # tail: mix(4) with pv(4) transposes woven in as exp2 coverage allows,
        # then the remaining pv/wt/proj
        zi_box = [None]
        zi_chain(zt_ks[4], zi_box)
        w_acc4 = bk.tile([P, 16, 48], bf, tag="wacc", bufs=2, name="wacc4")
        tpiece, mpiece, norm = make_pv(w_acc4)
        tp_sched = {4: 0, 8: 1, 12: 2, 16: 3}
        for j, piece in enumerate(mix_pieces(4, zi_box)):
            piece()
            if j in tp_sched:
                tpiece(0, tp_sched[j])
                tpiece(1, tp_sched[j])
                tpiece(2, tp_sched[j])
        for og in (0, 1, 2):
            tpiece(og, 4)
        for og in (0, 1, 2):
            for oj in range(4):
                mpiece(og, oj)
            norm(og)
        for og in (3,):
            for mt in range(5):
                tpiece(og, mt)
            for oj in range(4):
                mpiece(og, oj)
            norm(og)
        for piece in wt_pieces(4, w_acc4):
            piece()
        for piece in proj_pieces(4):
            piece()

        db.release()
        ld2.release()
        ps.release()
        bk.release()
        wp.release()
    nc.finalize()
    return nc


def _prep_weights(qkv_w, qkv_b, conv_w, proj_w, proj_b):
    f = np.float32
    qkv_w, qkv_b = qkv_w.astype(f), qkv_b.astype(f)
    wq = np.zeros((768, 1024), f)
    wk = np.zeros((768, 1024), f)
    wv = np.zeros((896, 784), f)
    bq = np.zeros((P, 8), f)
    for h in range(H):
        sl = slice(48 * h, 48 * h + 48)
        wq[:, 64 * h:64 * h + 48] = qkv_w[sl, :].T
        wk[:, 64 * h:64 * h + 48] = qkv_w[768 + 48 * h:768 + 48 * h + 48, :].T
        wv[:768, 49 * h:49 * h + 48] = qkv_w[1536 + 48 * h:1536 + 48 * h + 48, :].T
        wv[768, 49 * h:49 * h + 48] = qkv_b[1536 + 48 * h:1536 + 48 * h + 48]
        wv[768, 49 * h + 48] = 1.0
        for j in range(48):
            c64 = 64 * h + j
            bq[c64 % P, c64 // P] = qkv_b[48 * h + j]
    pwm = np.zeros((896, D), f)
    pwm[:768] = proj_w.astype(f).T
    pwm[768] = proj_b.astype(f)
    wblk = np.zeros((128, 64), f)
    for mj in range(4):
        wblk[32 * mj:32 * mj + 32, 16 * mj:16 * mj + 16] = conv_w.astype(f).T
    idn = np.eye(128, dtype=f)
    return {"wqT": wq.astype(BF), "wkT": wk.astype(BF), "wvT": wv.astype(BF),
            "pwT": pwm.astype(BF), "wblk": wblk.astype(BF), "idn": idn.astype(BF),
            "bqT": bq}


def kernel(x, x_freq, qkv_w, qkv_b, conv_w, conv_b, proj_w, proj_b, _profile=False):
    # conv_b is constant along the softmax axis -> cancels in softmax; unused.
    if "nc" not in _cache:
        _cache["nc"] = _build(bacc.Bacc())
    nc = _cache["nc"]
    wmap = _prep_weights(np.asarray(qkv_w), np.asarray(qkv_b), np.asarray(conv_w),
                         np.asarray(proj_w), np.asarray(proj_b))
    B = x.shape[0]
    in_maps = []
    for b in range(B):
        xT = np.zeros((896, NT), np.float32)
        xT[:768] = np.asarray(x[b], np.float32).T
        xT[768] = 1.0
        xfT = np.zeros((896, NT), np.float32)
        xfT[:768] = np.asarray(x_freq[b], np.float32).T
        xfT[768] = 1.0
        in_maps.append({"xT": xT.astype(BF), "xfT": xfT.astype(BF), **wmap})
    res = run_bass_kernel_spmd(nc, in_maps, core_ids=list(range(B)), trace=_profile)
    out = np.stack([res.results[b]["outT"].T for b in range(B)], axis=0)
    if _profile:
        return out.astype(np.float32), res
    return out.astype(np.float32)
